# revision 6
# baseline (speedup 1.0000x reference)
"""Trainium2 Bass kernel for a no-softmax attention head.

Reference computation (per batch element b, S=2048, DIN=1024, DQ=DK=128):
    Q = query @ Wq + bq;  K = key @ Wk + bk;  V = value @ Wv + bv
    out = (Q / sqrt(DQ)) @ (K^T @ V)

Sharding: batch dim across the 8 cores (B=8 -> 1 element/core), no collectives.

Per-core dataflow (compute dtype bf16 by default; fp32r fallback via
KERNEL_MODE=f32r):
  - Phase K: key/value stream through first. key tiles are PE-transposed
    per 128x128 chunk into [DIN, s] layout, projected to K^T (+bk), re-
    transposed to K [s, DK], and C = K^T @ value accumulates in PSUM with
    value used NATURALLY: the reassociation KtV = (K^T value) Wv +
    colsum(K) bv^T means value never needs a transpose.
  - KtV = C @ Wv (via PE-transposed C chunks) + colsum(K) x bv.
  - Phase Q: query streams last; per s-block the projection Q^T (+scale/bq
    folded into Wq/bq on the host) feeds the final out = Q^T.T @ KtV
    matmuls immediately, so stores overlap the remaining query work.
  - In bf16 mode the fp32->bf16 input casts ride inside GPSIMD (SWDGE)
    DMAs; weights are cast on DVE. fp32r mode loads via HWDGE untouched.
"""

import os
import sys

for _p in ("/opt/trn_rl_repo", "/root/.axon_site/_ro/trn_rl_repo"):
    if _p not in sys.path:
        sys.path.insert(0, _p)

import numpy as np

import concourse.mybir as mybir
import concourse.tile as tile
from concourse import bacc
from concourse.bass_utils import run_bass_kernel_spmd
from concourse.masks import make_identity

B, S, DIN, DQ, DK = 8, 2048, 1024, 128, 128
P = 128
NCH = DIN // P  # 8 din chunks
N_STILES = S // P  # 16
SBLOCK = 512
N_SBLOCKS = S // SBLOCK  # 4
TPB = SBLOCK // P  # 4

F32 = mybir.dt.float32
F32R = mybir.dt.float32r
BF16 = mybir.dt.bfloat16

MODE = os.environ.get("KERNEL_MODE", "bf16")


def _build_nc(mode=None):
    mode = mode or MODE
    cast_on_load = mode == "bf16"
    CD = BF16 if mode == "bf16" else F32R  # matmul operand dtype
    TD = BF16 if mode == "bf16" else F32  # transpose path dtype
    tpg = 8 if TD == BF16 else 4  # transposes per PSUM bank

    nc = bacc.Bacc("TRN2", target_bir_lowering=False, debug=False, num_devices=8)

    def dram_in(name, shape, used_by_matmul):
        dt = F32 if (cast_on_load or not used_by_matmul) else F32R
        return nc.declare_dram_parameter(name, shape, dt, isOutput=False)

    q_d = dram_in("query", [S, DIN], False)
    k_d = dram_in("key", [S, DIN], False)
    v_d = dram_in("value", [S, DIN], True)
    wq_d = dram_in("Wq", [DIN, DQ], True)
    wk_d = dram_in("Wk", [DIN, DK], True)
    wv_d = dram_in("Wv", [DIN, DK], True)
    bq_d = dram_in("bq", [DQ], False)
    bk_d = dram_in("bk", [DK], False)
    bv_d = dram_in("bv", [DK], True)
    out_d = nc.declare_dram_parameter("out", [S, DK], F32, isOutput=True)

    def load(dst, src_ap):
        if cast_on_load:
            nc.gpsimd.dma_start(out=dst, in_=src_ap)
        else:
            nc.sync.dma_start(out=dst, in_=src_ap)

    from contextlib import ExitStack

    with tile.TileContext(nc) as tc, ExitStack() as ctx:
        singles = ctx.enter_context(tc.tile_pool(name="singles", bufs=1))
        nat = ctx.enter_context(tc.tile_pool(name="nat", bufs=3))
        tposed = ctx.enter_context(tc.tile_pool(name="tposed", bufs=2))
        kslab = ctx.enter_context(tc.tile_pool(name="kslab", bufs=2))
        qblk = ctx.enter_context(tc.tile_pool(name="qblk", bufs=2))
        psum_t = ctx.enter_context(tc.tile_pool(name="psum_t", bufs=2, space="PSUM"))
        psum_p = ctx.enter_context(tc.tile_pool(name="psum_p", bufs=2, space="PSUM"))
        psum_c = ctx.enter_context(tc.tile_pool(name="psum_c", bufs=1, space="PSUM"))
        psum_o = ctx.enter_context(tc.tile_pool(name="psum_o", bufs=2, space="PSUM"))
        outsb = ctx.enter_context(tc.tile_pool(name="outsb", bufs=4))

        # identity first: it gates every transpose and shares the GpSimd
        # sequencer with the SWDGE cast loads.
        ident = singles.tile([P, P], TD)
        make_identity(nc, ident[:])

        # weights: HWDGE load (+ DVE cast in bf16 mode)
        if cast_on_load:
            w_raw = singles.tile([P, 3, NCH, DQ], F32)
            nc.sync.dma_start(
                out=w_raw[:, 0], in_=wk_d.ap().rearrange("(c p) d -> p c d", p=P)
            )
            nc.sync.dma_start(
                out=w_raw[:, 1], in_=wv_d.ap().rearrange("(c p) d -> p c d", p=P)
            )
            nc.sync.dma_start(
                out=w_raw[:, 2], in_=wq_d.ap().rearrange("(c p) d -> p c d", p=P)
            )
            wk_sb = singles.tile([P, NCH, DK], CD)
            wv_sb = singles.tile([P, NCH, DK], CD)
            wq_sb = singles.tile([P, NCH, DQ], CD)
            nc.vector.tensor_copy(wk_sb[:], w_raw[:, 0])
            nc.vector.tensor_copy(wv_sb[:], w_raw[:, 1])
            nc.vector.tensor_copy(wq_sb[:], w_raw[:, 2])
        else:
            wk_sb = singles.tile([P, NCH, DK], CD)
            wv_sb = singles.tile([P, NCH, DK], CD)
            wq_sb = singles.tile([P, NCH, DQ], CD)
            nc.sync.dma_start(
                out=wk_sb, in_=wk_d.ap().rearrange("(c p) d -> p c d", p=P)
            )
            nc.sync.dma_start(
                out=wv_sb, in_=wv_d.ap().rearrange("(c p) d -> p c d", p=P)
            )
            nc.sync.dma_start(
                out=wq_sb, in_=wq_d.ap().rearrange("(c p) d -> p c d", p=P)
            )

        bq_col = singles.tile([P, 1], F32)
        bk_col = singles.tile([P, 1], F32)
        bv_row = singles.tile([1, DK], CD)
        nc.sync.dma_start(out=bq_col, in_=bq_d.ap().unsqueeze(1))
        nc.sync.dma_start(out=bk_col, in_=bk_d.ap().unsqueeze(1))
        load(bv_row, bv_d.ap().unsqueeze(0))

        kcol = singles.tile([P, 1], F32)
        nc.vector.memset(kcol, 0.0)
        c_ps = psum_c.tile([P, DIN], F32)  # C = K^T @ value (2 banks, pinned)

        def transpose_tile_chunks(nat_tile, slab, t, n_chunks, alt):
            """PE-transpose n_chunks [128,128] chunks of nat_tile into
            slab[:, c, t*P:(t+1)*P], batching tpg chunks per PSUM bank."""
            for g in range(n_chunks // tpg):
                ps = psum_t.tile([P, tpg * P], TD, tag="tp")
                for j in range(tpg):
                    c = g * tpg + j
                    nc.tensor.transpose(
                        ps[:, j * P : (j + 1) * P],
                        nat_tile[:, c * P : (c + 1) * P],
                        ident[:],
                    )
                dst = slab[:, g * tpg : (g + 1) * tpg, t * P : (t + 1) * P]
                src = ps[:].rearrange("p (j s) -> p j s", j=tpg)
                if alt % 2 == 0:
                    nc.vector.tensor_copy(dst, src)
                else:
                    nc.scalar.activation(dst, src, mybir.ActivationFunctionType.Copy)

        # ================= Phase K: key + value -> C =================
        for blk in range(N_SBLOCKS):
            kt_slab = tposed.tile([P, NCH, SBLOCK], CD, tag="kt")
            for t in range(TPB):
                st = blk * TPB + t
                k_nat = nat.tile([P, DIN], TD, tag="k_nat")
                load(k_nat, k_d.ap()[st * P : (st + 1) * P, :])
                transpose_tile_chunks(k_nat, kt_slab, t, NCH, st)

            kp = psum_p.tile([P, SBLOCK], F32, tag="proj")
            for c in range(NCH):
                nc.tensor.matmul(
                    kp[:],
                    wk_sb[:, c, :],
                    kt_slab[:, c, :],
                    start=(c == 0),
                    stop=(c == NCH - 1),
                )

            kt_sb = kslab.tile([P, SBLOCK], TD, tag="ktsb")
            nc.scalar.activation(
                kt_sb[:], kp[:], mybir.ActivationFunctionType.Identity, bias=bk_col[:]
            )
            kc_part = kslab.tile([P, 1], F32, tag="kcp")
            nc.vector.reduce_sum(kc_part[:], kt_sb[:], axis=mybir.AxisListType.X)
            nc.vector.tensor_add(out=kcol[:], in0=kcol[:], in1=kc_part[:])

            k_slab = kslab.tile([P, TPB, DK], CD, tag="kslab")
            ps_k = psum_t.tile([P, tpg * P], TD, tag="tp")
            for t in range(TPB):
                nc.tensor.transpose(
                    ps_k[:, t * P : (t + 1) * P],
                    kt_sb[:, t * P : (t + 1) * P],
                    ident[:],
                )
            nc.vector.tensor_copy(
                k_slab[:], ps_k[:, : TPB * P].rearrange("p (t d) -> p t d", t=TPB)
            )

            for t in range(TPB):
                st = blk * TPB + t
                v_nat = nat.tile([P, DIN], CD, tag="v_nat")
                load(v_nat, v_d.ap()[st * P : (st + 1) * P, :])
                for h in range(2):
                    nc.tensor.matmul(
                        c_ps[:, h * SBLOCK : (h + 1) * SBLOCK],
                        k_slab[:, t, :],
                        v_nat[:, h * SBLOCK : (h + 1) * SBLOCK],
                        start=(st == 0),
                        stop=(st == N_STILES - 1),
                    )

        # ================= KtV = C @ Wv + colsum(K) x bv =================
        c_sb = singles.tile([P, DIN], TD)
        nc.vector.tensor_copy(c_sb[:], c_ps[:])

        ct_sb = singles.tile([P, NCH, DK], CD)
        for g in range(NCH // tpg):
            ps = psum_t.tile([P, tpg * P], TD, tag="tp")
            for j in range(tpg):
                c = g * tpg + j
                nc.tensor.transpose(
                    ps[:, j * P : (j + 1) * P], c_sb[:, c * P : (c + 1) * P], ident[:]
                )
            nc.vector.tensor_copy(
                ct_sb[:, g * tpg : (g + 1) * tpg, :],
                ps[:].rearrange("p (j d) -> p j d", j=tpg),
            )

        if TD != F32:
            kcol_td = singles.tile([P, 1], TD)
            nc.vector.tensor_copy(kcol_td[:], kcol[:])
            kcol_src = kcol_td
        else:
            kcol_src = kcol
        kcol_t_bank = psum_o.tile([P, DK], TD, tag="po")
        kcol_t_ps = kcol_t_bank[:1, :]
        nc.tensor.transpose(kcol_t_ps, kcol_src[:], ident[:])
        kcol_row = singles.tile([1, P], CD)
        nc.vector.tensor_copy(kcol_row[:], kcol_t_ps)

        ktv_ps = psum_o.tile([P, DK], F32, tag="po")
        for c in range(NCH):
            nc.tensor.matmul(
                ktv_ps[:], ct_sb[:, c, :], wv_sb[:, c, :], start=(c == 0), stop=False
            )
        nc.tensor.matmul(ktv_ps[:], kcol_row[:], bv_row[:], start=False, stop=True)
        ktv_sb = singles.tile([P, DK], CD)
        nc.vector.tensor_copy(ktv_sb[:], ktv_ps[:])

        # ================= Phase Q: query -> out =================
        for blk in range(N_SBLOCKS):
            qt_slab = tposed.tile([P, NCH, SBLOCK], CD, tag="kt")
            for t in range(TPB):
                st = blk * TPB + t
                q_nat = nat.tile([P, DIN], TD, tag="k_nat")
                load(q_nat, q_d.ap()[st * P : (st + 1) * P, :])
                transpose_tile_chunks(q_nat, qt_slab, t, NCH, st)

            qp = psum_p.tile([P, SBLOCK], F32, tag="proj")
            for c in range(NCH):
                nc.tensor.matmul(
                    qp[:],
                    wq_sb[:, c, :],
                    qt_slab[:, c, :],
                    start=(c == 0),
                    stop=(c == NCH - 1),
                )
            qt_blk = qblk.tile([P, SBLOCK], CD, tag="qtb")
            nc.vector.tensor_scalar_add(out=qt_blk[:], in0=qp[:], scalar1=bq_col[:])

            for t in range(TPB):
                st = blk * TPB + t
                po = psum_o.tile([P, DK], F32, tag="po")
                nc.tensor.matmul(
                    po[:],
                    qt_blk[:, t * P : (t + 1) * P],
                    ktv_sb[:],
                    start=True,
                    stop=True,
                )
                o_sb = outsb.tile([P, DK], F32, tag="osb")
                if st % 2 == 0:
                    nc.vector.tensor_copy(o_sb[:], po[:])
                else:
                    nc.scalar.activation(
                        o_sb[:], po[:], mybir.ActivationFunctionType.Copy
                    )
                nc.sync.dma_start(
                    out=out_d.ap()[st * P : (st + 1) * P, :], in_=o_sb[:]
                )

    nc.compile()
    return nc


_NC_CACHE = {}


def _get_nc(mode=None):
    mode = mode or MODE
    if mode not in _NC_CACHE:
        _NC_CACHE[mode] = _build_nc(mode)
    return _NC_CACHE[mode]


def kernel(query, key, value, Wq, bq, Wk, bk, Wv, bv, **_ignored):
    query = np.ascontiguousarray(np.asarray(query, dtype=np.float32))
    key = np.ascontiguousarray(np.asarray(key, dtype=np.float32))
    value = np.ascontiguousarray(np.asarray(value, dtype=np.float32))
    scale = np.float32(1.0 / np.sqrt(np.float32(DQ)))
    wq_s = np.ascontiguousarray(np.asarray(Wq, dtype=np.float32) * scale)
    bq_s = np.ascontiguousarray(np.asarray(bq, dtype=np.float32) * scale)
    wk = np.ascontiguousarray(np.asarray(Wk, dtype=np.float32))
    bk_ = np.ascontiguousarray(np.asarray(bk, dtype=np.float32))
    wv = np.ascontiguousarray(np.asarray(Wv, dtype=np.float32))
    bv_ = np.ascontiguousarray(np.asarray(bv, dtype=np.float32))

    nc = _get_nc()
    in_maps = [
        {
            "query": query[b],
            "key": key[b],
            "value": value[b],
            "Wq": wq_s,
            "Wk": wk,
            "Wv": wv,
            "bq": bq_s,
            "bk": bk_,
            "bv": bv_,
        }
        for b in range(B)
    ]
    res = run_bass_kernel_spmd(nc, in_maps, list(range(B)))
    return np.stack([res.results[b]["out"] for b in range(B)], axis=0)


if __name__ == "__main__":
    rng = np.random.default_rng(0)
    inputs = {
        "query": rng.standard_normal((B, S, DIN), dtype=np.float32),
        "key": rng.standard_normal((B, S, DIN), dtype=np.float32),
        "value": rng.standard_normal((B, S, DIN), dtype=np.float32),
        "Wq": (rng.standard_normal((DIN, DQ), dtype=np.float32) * 0.02),
        "bq": rng.standard_normal((DQ,), dtype=np.float32) * 0.1,
        "Wk": (rng.standard_normal((DIN, DK), dtype=np.float32) * 0.02),
        "bk": rng.standard_normal((DK,), dtype=np.float32) * 0.1,
        "Wv": (rng.standard_normal((DIN, DK), dtype=np.float32) * 0.02),
        "bv": rng.standard_normal((DK,), dtype=np.float32) * 0.1,
    }
    out = kernel(**inputs)

    def ref(query, key, value, Wq, bq, Wk, bk, Wv, bv):
        Q = query.astype(np.float64) @ Wq.astype(np.float64) + bq
        K = key.astype(np.float64) @ Wk.astype(np.float64) + bk
        V = value.astype(np.float64) @ Wv.astype(np.float64) + bv
        scale = 1.0 / np.sqrt(np.float64(Q.shape[-1]))
        KtV = np.einsum("bsk,bsv->bkv", K, V)
        return (Q * scale) @ KtV

    expected = ref(**inputs)
    err = np.abs(out - expected).max() / np.abs(expected).max()
    print("max out:", np.abs(out).max(), "rel err:", err)


# revision 7
# speedup vs baseline: 1.2375x; 1.2375x over previous
"""Trainium2 Bass kernel for a no-softmax attention head.

Reference computation (per batch element b, S=2048, DIN=1024, DQ=DK=128):
    Q = query @ Wq + bq;  K = key @ Wk + bk;  V = value @ Wv + bv
    out = (Q / sqrt(DQ)) @ (K^T @ V)

Sharding: batch dim across the 8 cores (B=8 -> 1 element/core), no collectives.

Per-core dataflow (compute dtype bf16 by default; fp32r fallback):
  - query/key/value loaded naturally [s=128, DIN]; in bf16 mode the fp32->bf16
    cast happens inside the GPSIMD (SWDGE) DMA, so no compute engine pays for
    it.
  - query/key tiles are PE-transposed per 128x128 chunk into [DIN, s] layout
    (chunks batched per PSUM bank, one wide evacuation each).
  - Q^T [DQ, S] = Wq-chunk matmuls with 512-wide moving operands; scale and bq
    are folded into Wq/bq on the host.
  - K^T likewise (+bk), then re-transposed per 128-chunk to K [s, DK].
  - C = K^T @ value [DK, DIN] accumulates in PSUM with value tiles used
    NATURALLY: the reassociation KtV = (K^T value) Wv + colsum(K) bv^T avoids
    transposing value at all.
  - KtV = C @ Wv (via PE-transposed C chunks) + colsum(K) x bv.
  - out tile t = (Q^T[:, t])^T @ KtV, stored naturally in fp32.
"""

import os
import sys

for _p in ("/opt/trn_rl_repo", "/root/.axon_site/_ro/trn_rl_repo"):
    if _p not in sys.path:
        sys.path.insert(0, _p)

import numpy as np

import concourse.mybir as mybir
import concourse.tile as tile
from concourse import bacc
from concourse.bass_utils import run_bass_kernel_spmd
from concourse.masks import make_identity

B, S, DIN, DQ, DK = 8, 2048, 1024, 128, 128
P = 128  # partition size / tile edge
NCH = DIN // P  # 8 din chunks
N_STILES = S // P  # 16 s-tiles per core
SBLOCK = 512  # moving-operand width for projections
N_SBLOCKS = S // SBLOCK  # 4
TPB = SBLOCK // P  # s-tiles per block: 4

F32 = mybir.dt.float32
F32R = mybir.dt.float32r
BF16 = mybir.dt.bfloat16

# Compute mode: "bf16" (fast, ~5e-3 rel err) or "f32r" (~3e-4 rel err).
MODE = os.environ.get("KERNEL_MODE", "bf16")


def _build_nc(mode=None):
    mode = mode or MODE
    cast_on_load = mode == "bf16"
    CD = BF16 if mode == "bf16" else F32R  # matmul operand dtype
    TD = BF16 if mode == "bf16" else F32  # transpose path dtype
    # transposes batched per PSUM bank (bank = 2KB/partition): 8 or 4
    tpg = 2048 // (2 * P) if TD == BF16 else 2048 // (4 * P)

    nc = bacc.Bacc("TRN2", target_bir_lowering=False, debug=False, num_devices=8)

    def dram_in(name, shape, used_by_matmul):
        dt = F32 if (cast_on_load or not used_by_matmul) else F32R
        return nc.declare_dram_parameter(name, shape, dt, isOutput=False)

    q_d = dram_in("query", [S, DIN], False)
    k_d = dram_in("key", [S, DIN], False)
    v_d = dram_in("value", [S, DIN], True)
    wq_d = dram_in("Wq", [DIN, DQ], True)
    wk_d = dram_in("Wk", [DIN, DK], True)
    wv_d = dram_in("Wv", [DIN, DK], True)
    bq_d = dram_in("bq", [DQ], False)
    bk_d = dram_in("bk", [DK], False)
    bv_d = dram_in("bv", [DK], True)
    out_d = nc.declare_dram_parameter("out", [S, DK], F32, isOutput=True)

    def load(dst, src_ap):
        if cast_on_load:
            nc.gpsimd.dma_start(out=dst, in_=src_ap)
        else:
            nc.sync.dma_start(out=dst, in_=src_ap)

    from contextlib import ExitStack

    with tile.TileContext(nc) as tc, ExitStack() as ctx:
        singles = ctx.enter_context(tc.tile_pool(name="singles", bufs=1))
        nat = ctx.enter_context(tc.tile_pool(name="nat", bufs=6))
        vnat = ctx.enter_context(tc.tile_pool(name="vnat", bufs=4))
        tposed = ctx.enter_context(tc.tile_pool(name="tposed", bufs=2))
        kslab = ctx.enter_context(tc.tile_pool(name="kslab", bufs=2))
        psum_t = ctx.enter_context(tc.tile_pool(name="psum_t", bufs=2, space="PSUM"))
        psum_p = ctx.enter_context(tc.tile_pool(name="psum_p", bufs=2, space="PSUM"))
        psum_c = ctx.enter_context(tc.tile_pool(name="psum_c", bufs=1, space="PSUM"))
        psum_o = ctx.enter_context(tc.tile_pool(name="psum_o", bufs=2, space="PSUM"))
        outsb = ctx.enter_context(tc.tile_pool(name="outsb", bufs=4))

        # ---- constants / weights ----
        ident = singles.tile([P, P], TD)
        make_identity(nc, ident[:])

        wq_sb = singles.tile([P, NCH, DQ], CD)
        wk_sb = singles.tile([P, NCH, DK], CD)
        wv_sb = singles.tile([P, NCH, DK], CD)
        if cast_on_load:
            w_raw = singles.tile([P, 3, NCH, DQ], F32)
            nc.sync.dma_start(
                out=w_raw[:, 0], in_=wq_d.ap().rearrange("(c p) d -> p c d", p=P)
            )
            nc.sync.dma_start(
                out=w_raw[:, 1], in_=wk_d.ap().rearrange("(c p) d -> p c d", p=P)
            )
            nc.sync.dma_start(
                out=w_raw[:, 2], in_=wv_d.ap().rearrange("(c p) d -> p c d", p=P)
            )
            nc.vector.tensor_copy(wq_sb[:], w_raw[:, 0])
            nc.vector.tensor_copy(wk_sb[:], w_raw[:, 1])
            nc.vector.tensor_copy(wv_sb[:], w_raw[:, 2])
        else:
            nc.sync.dma_start(out=wq_sb, in_=wq_d.ap().rearrange("(c p) d -> p c d", p=P))
            nc.sync.dma_start(out=wk_sb, in_=wk_d.ap().rearrange("(c p) d -> p c d", p=P))
            nc.sync.dma_start(out=wv_sb, in_=wv_d.ap().rearrange("(c p) d -> p c d", p=P))

        bq_col = singles.tile([P, 1], F32)
        bk_col = singles.tile([P, 1], F32)
        bv_row = singles.tile([1, DK], CD)
        nc.sync.dma_start(out=bq_col, in_=bq_d.ap().unsqueeze(1))
        nc.sync.dma_start(out=bk_col, in_=bk_d.ap().unsqueeze(1))
        load(bv_row, bv_d.ap().unsqueeze(0))

        # ---- persistent intermediates ----
        qt_full = singles.tile([P, S], CD)  # Q^T [DQ, S] (scale+bq folded)
        kcol = singles.tile([P, 1], F32)  # colsum of K over s
        nc.vector.memset(kcol, 0.0)
        c_ps = psum_c.tile([P, DIN], F32)  # C = K^T @ value, 2 banks, pinned

        for blk in range(N_SBLOCKS):
            # transposed input slabs for this s-block: [p, chunk, s_in_block]
            qt_slab = tposed.tile([P, NCH, SBLOCK], CD, tag="qt")
            kt_slab = tposed.tile([P, NCH, SBLOCK], CD, tag="kt")

            for t in range(TPB):
                st = blk * TPB + t  # global s-tile index
                s0 = st * P

                q_nat = nat.tile([P, DIN], TD, tag="q_nat")
                k_nat = nat.tile([P, DIN], TD, tag="k_nat")
                load(q_nat, q_d.ap()[s0 : s0 + P, :])
                load(k_nat, k_d.ap()[s0 : s0 + P, :])

                # PE-transpose the 8 [128,128] chunks of each tile; batch tpg
                # chunks per PSUM bank, evacuate [128, tpg*P] at a time.
                for nat_tile, slab in ((q_nat, qt_slab), (k_nat, kt_slab)):
                    for g in range(NCH // tpg):
                        ps = psum_t.tile([P, tpg * P], TD, tag="tp")
                        for j in range(tpg):
                            c = g * tpg + j
                            nc.tensor.transpose(
                                ps[:, j * P : (j + 1) * P],
                                nat_tile[:, c * P : (c + 1) * P],
                                ident[:],
                            )
                        dst = slab[:, g * tpg : (g + 1) * tpg, t * P : (t + 1) * P]
                        src = ps[:].rearrange("p (j s) -> p j s", j=tpg)
                        if (st + g) % 2 == 0:
                            nc.vector.tensor_copy(dst, src)
                        else:
                            nc.scalar.activation(
                                dst, src, mybir.ActivationFunctionType.Copy
                            )

            # ---- projections for this s-block (moving dim = SBLOCK) ----
            qp = psum_p.tile([P, SBLOCK], F32, tag="proj")
            kp = psum_p.tile([P, SBLOCK], F32, tag="proj")
            for c in range(NCH):
                nc.tensor.matmul(
                    qp[:],
                    wq_sb[:, c, :],
                    qt_slab[:, c, :],
                    start=(c == 0),
                    stop=(c == NCH - 1),
                )
            for c in range(NCH):
                nc.tensor.matmul(
                    kp[:],
                    wk_sb[:, c, :],
                    kt_slab[:, c, :],
                    start=(c == 0),
                    stop=(c == NCH - 1),
                )

            # Q^T evac (+ folded bias) straight into persistent buffer
            nc.vector.tensor_scalar_add(
                out=qt_full[:, blk * SBLOCK : (blk + 1) * SBLOCK],
                in0=qp[:],
                scalar1=bq_col[:],
            )

            # K^T evac (+bk) then per-chunk re-transpose to K [s, DK]
            kt_sb = kslab.tile([P, SBLOCK], TD, tag="ktsb")
            nc.scalar.activation(
                kt_sb[:],
                kp[:],
                mybir.ActivationFunctionType.Identity,
                bias=bk_col[:],
            )
            # colsum of K over s for the bv correction term
            kc_part = kslab.tile([P, 1], F32, tag="kcp")
            nc.vector.reduce_sum(kc_part[:], kt_sb[:], axis=mybir.AxisListType.X)
            nc.vector.tensor_add(out=kcol[:], in0=kcol[:], in1=kc_part[:])

            k_slab = kslab.tile([P, TPB, DK], CD, tag="kslab")
            ps_k = psum_t.tile([P, tpg * P], TD, tag="tp")
            for t in range(TPB):
                nc.tensor.transpose(
                    ps_k[:, t * P : (t + 1) * P],
                    kt_sb[:, t * P : (t + 1) * P],
                    ident[:],
                )
            nc.vector.tensor_copy(
                k_slab[:],
                ps_k[:, : TPB * P].rearrange("p (t d) -> p t d", t=TPB),
            )

            # ---- C += K_t^T @ value_t for the tiles of this block ----
            for t in range(TPB):
                st = blk * TPB + t
                s0 = st * P
                if cast_on_load:
                    v_raw = vnat.tile([P, DIN], F32, tag="v_raw")
                    nc.sync.dma_start(out=v_raw, in_=v_d.ap()[s0 : s0 + P, :])
                    v_nat = vnat.tile([P, DIN], CD, tag="v_nat")
                    if st % 2 == 0:
                        nc.vector.tensor_copy(v_nat[:], v_raw[:])
                    else:
                        nc.scalar.activation(
                            v_nat[:], v_raw[:], mybir.ActivationFunctionType.Copy
                        )
                else:
                    v_nat = vnat.tile([P, DIN], CD, tag="v_nat")
                    load(v_nat, v_d.ap()[s0 : s0 + P, :])
                for h in range(2):
                    nc.tensor.matmul(
                        c_ps[:, h * SBLOCK : (h + 1) * SBLOCK],
                        k_slab[:, t, :],
                        v_nat[:, h * SBLOCK : (h + 1) * SBLOCK],
                        start=(st == 0),
                        stop=(st == N_STILES - 1),
                    )

        # ---- KtV = C @ Wv + colsum(K) x bv ----
        c_sb = singles.tile([P, DIN], TD)
        nc.vector.tensor_copy(c_sb[:], c_ps[:])

        ct_sb = singles.tile([P, NCH, DK], CD)  # C^T chunks [din_c, DK]
        for g in range(NCH // tpg):
            ps = psum_t.tile([P, tpg * P], TD, tag="tp")
            for j in range(tpg):
                c = g * tpg + j
                nc.tensor.transpose(
                    ps[:, j * P : (j + 1) * P],
                    c_sb[:, c * P : (c + 1) * P],
                    ident[:],
                )
            nc.vector.tensor_copy(
                ct_sb[:, g * tpg : (g + 1) * tpg, :],
                ps[:].rearrange("p (j d) -> p j d", j=tpg),
            )

        # colsum(K) as a row vector [1, DK] via PE transpose
        if TD != F32:
            kcol_td = singles.tile([P, 1], TD)
            nc.vector.tensor_copy(kcol_td[:], kcol[:])
            kcol_src = kcol_td
        else:
            kcol_src = kcol
        kcol_t_bank = psum_o.tile([P, DK], TD, tag="po")
        kcol_t_ps = kcol_t_bank[:1, :]
        nc.tensor.transpose(kcol_t_ps, kcol_src[:], ident[:])
        kcol_row = singles.tile([1, P], CD)
        nc.vector.tensor_copy(kcol_row[:], kcol_t_ps)

        ktv_ps = psum_o.tile([P, DK], F32, tag="po")
        for c in range(NCH):
            nc.tensor.matmul(
                ktv_ps[:],
                ct_sb[:, c, :],
                wv_sb[:, c, :],
                start=(c == 0),
                stop=False,
            )
        nc.tensor.matmul(ktv_ps[:], kcol_row[:], bv_row[:], start=False, stop=True)
        ktv_sb = singles.tile([P, DK], CD)
        nc.vector.tensor_copy(ktv_sb[:], ktv_ps[:])

        # ---- out tile t = (Q^T[:, t*P:(t+1)*P])^T @ KtV ----
        for t in range(N_STILES):
            po = psum_o.tile([P, DK], F32, tag="po")
            nc.tensor.matmul(
                po[:],
                qt_full[:, t * P : (t + 1) * P],
                ktv_sb[:],
                start=True,
                stop=True,
            )
            o_sb = outsb.tile([P, DK], F32, tag="osb")
            nc.vector.tensor_copy(o_sb[:], po[:])
            nc.sync.dma_start(out=out_d.ap()[t * P : (t + 1) * P, :], in_=o_sb[:])

    nc.compile()
    return nc


_NC_CACHE = {}


def _get_nc(mode=None):
    mode = mode or MODE
    if mode not in _NC_CACHE:
        _NC_CACHE[mode] = _build_nc(mode)
    return _NC_CACHE[mode]


def kernel(query, key, value, Wq, bq, Wk, bk, Wv, bv, **_ignored):
    query = np.ascontiguousarray(np.asarray(query, dtype=np.float32))
    key = np.ascontiguousarray(np.asarray(key, dtype=np.float32))
    value = np.ascontiguousarray(np.asarray(value, dtype=np.float32))
    scale = np.float32(1.0 / np.sqrt(np.float32(DQ)))
    wq_s = np.ascontiguousarray(np.asarray(Wq, dtype=np.float32) * scale)
    bq_s = np.ascontiguousarray(np.asarray(bq, dtype=np.float32) * scale)
    wk = np.ascontiguousarray(np.asarray(Wk, dtype=np.float32))
    bk_ = np.ascontiguousarray(np.asarray(bk, dtype=np.float32))
    wv = np.ascontiguousarray(np.asarray(Wv, dtype=np.float32))
    bv_ = np.ascontiguousarray(np.asarray(bv, dtype=np.float32))

    nc = _get_nc()
    in_maps = [
        {
            "query": query[b],
            "key": key[b],
            "value": value[b],
            "Wq": wq_s,
            "Wk": wk,
            "Wv": wv,
            "bq": bq_s,
            "bk": bk_,
            "bv": bv_,
        }
        for b in range(B)
    ]
    res = run_bass_kernel_spmd(nc, in_maps, list(range(B)))
    return np.stack([res.results[b]["out"] for b in range(B)], axis=0)


if __name__ == "__main__":
    rng = np.random.default_rng(0)
    inputs = {
        "query": rng.standard_normal((B, S, DIN), dtype=np.float32),
        "key": rng.standard_normal((B, S, DIN), dtype=np.float32),
        "value": rng.standard_normal((B, S, DIN), dtype=np.float32),
        "Wq": (rng.standard_normal((DIN, DQ), dtype=np.float32) * 0.02),
        "bq": rng.standard_normal((DQ,), dtype=np.float32) * 0.1,
        "Wk": (rng.standard_normal((DIN, DK), dtype=np.float32) * 0.02),
        "bk": rng.standard_normal((DK,), dtype=np.float32) * 0.1,
        "Wv": (rng.standard_normal((DIN, DK), dtype=np.float32) * 0.02),
        "bv": rng.standard_normal((DK,), dtype=np.float32) * 0.1,
    }
    out = kernel(**inputs)

    def ref(query, key, value, Wq, bq, Wk, bk, Wv, bv):
        Q = query.astype(np.float64) @ Wq.astype(np.float64) + bq
        K = key.astype(np.float64) @ Wk.astype(np.float64) + bk
        V = value.astype(np.float64) @ Wv.astype(np.float64) + bv
        scale = 1.0 / np.sqrt(np.float64(Q.shape[-1]))
        KtV = np.einsum("bsk,bsv->bkv", K, V)
        return (Q * scale) @ KtV

    expected = ref(**inputs)
    err = np.abs(out - expected).max() / np.abs(expected).max()
    print("max out:", np.abs(out).max(), "rel err:", err)
